# revision 11
# baseline (speedup 1.0000x reference)
"""Trainium2 Bass kernel for nn_MoEBlock_64733747085415.

MoE block: 8 experts (top-2 routed combine) + shared expert, on
B*S = 4096 tokens, D = 1024, I = 4096.

Strategy (expert-parallel with host-side token routing):
  - The gate (softmax over x @ gate_w.T, top-2) is evaluated on the host in
    exact fp32; it only steers the sharding.  Each token's combine weight is
    zero for the 6 unselected experts, so each core only computes its own
    expert's FFN on the ~N*k/E tokens that actually routed to it (gathered,
    zero-padded to capacity C), a 4x FLOP reduction over the dense broadcast.
  - The shared expert is tensor-parallel on the inner dim: core c owns
    i-slice [512c, 512c+512) and processes ALL 4096 tokens; the host sums
    the 8 partial outputs (exact: gelu is elementwise over i).
  - FFN runs feature-major in fp16 (PSUM accumulates fp32): h^T tiles =
    gelu(w1_tile.T @ x^T + b1), y^T tiles = w2_tile.T @ h^T.  Biases b2/s_b2
    and the weighted top-2 combine are applied on the host in fp32.
  - No collectives and no gate on device; the tensor engine runs a pure
    matmul stream (~2048 matmuls/core).

Phase order per core: A-expert, A-shared(half 0), B-expert, A-shared(half 1),
B-shared - so the second half of the replicated x^T streams in behind
B-expert's compute instead of stalling the PE.
"""

import sys
import types

import numpy as np

import concourse.bass as bass
import concourse.mybir as mybir
import concourse.tile as tile
from concourse import bacc
from concourse import bass_utils

F32 = mybir.dt.float32
F16 = mybir.dt.float16

N_CORES = 8
N = 4096          # tokens
D = 1024          # model dim
I = 4096          # expert inner dim
E = 8             # experts
IS = I // N_CORES  # shared-expert inner slice per core (512)
DT = D // 128     # 8 d-tiles
IT_E = I // 128   # 32 expert i-tiles
IT_S = IS // 128  # 4 shared i-tiles
NH = N // 2       # shared-x half (2048 tokens)

_NC_CACHE = {}


def install_ntff_hook():
    """Register the axon NTFF profile hook that boot skips when the antenv
    stub lacks axon_hooks.  Needed only for trace=True runs."""
    if "antenv.axon_hooks" in sys.modules:
        return
    try:
        import trn_agent_boot.trn_boot as tb

        hook = tb._ntff_profile_via_ctypes("/opt/axon/libaxon_pjrt.so")
    except Exception:
        return
    mod = types.ModuleType("antenv.axon_hooks")
    mod.get_axon_ntff_profile_hook = lambda: hook
    mod.set_axon_ntff_profile_hook = lambda h: None
    sys.modules["antenv.axon_hooks"] = mod
    import antenv

    antenv.axon_hooks = mod
    bass_utils.upload_artifacts = lambda tmpdir: tmpdir


def _chunks(total):
    """Split a token count into matmul free-dim chunks of <=512."""
    out = []
    off = 0
    while off < total:
        ch = min(512, total - off)
        out.append((off, ch))
        off += ch
    return out


def build_nc(C):
    nc = bacc.Bacc(
        "TRN2", target_bir_lowering=False, debug=False, num_devices=N_CORES
    )

    # ---- kernel I/O (per-core) ----
    xg_d = nc.dram_tensor("xg", [128, DT, C], F16, kind="ExternalInput")
    xsh_d = nc.dram_tensor("xsh", [2, 128, DT, NH], F16, kind="ExternalInput")
    w1t_d = nc.dram_tensor("w1t", [IT_E, 128, DT, 128], F16, kind="ExternalInput")
    w2t_d = nc.dram_tensor("w2t", [DT, 128, IT_E, 128], F16, kind="ExternalInput")
    s1t_d = nc.dram_tensor("s1t", [IT_S, 128, DT, 128], F16, kind="ExternalInput")
    s2t_d = nc.dram_tensor("s2t", [DT, 128, IT_S, 128], F16, kind="ExternalInput")
    b1_d = nc.dram_tensor("b1c", [128, IT_E], F32, kind="ExternalInput")
    sb1_d = nc.dram_tensor("sb1c", [128, IT_S], F32, kind="ExternalInput")
    ye_d = nc.dram_tensor("ye", [DT, 128, C], F16, kind="ExternalOutput")
    ys_d = nc.dram_tensor("ys", [DT, 128, N], F16, kind="ExternalOutput")

    ech = _chunks(C)    # expert token chunks
    sch = _chunks(NH)   # shared-half token chunks (4 x 512)

    with tile.TileContext(nc) as tc:
        with (
            tc.tile_pool(name="const", bufs=1) as cpool,
            tc.tile_pool(name="xgp", bufs=1) as xg_pool,
            tc.tile_pool(name="xsp", bufs=1) as xs_pool,
            tc.tile_pool(name="hep", bufs=1) as he_pool,
            tc.tile_pool(name="hsp", bufs=1) as hs_pool,
            tc.tile_pool(name="w1s", bufs=6) as w1_pool,
            tc.tile_pool(name="w2s", bufs=2) as w2_pool,
            tc.tile_pool(name="s2s", bufs=2) as s2_pool,
            tc.tile_pool(name="yb", bufs=2) as y_pool,
            tc.tile_pool(name="aps", bufs=4, space="PSUM") as aps,
            tc.tile_pool(name="bps", bufs=4, space="PSUM") as bps,
        ):
            # issue the first weight tile and xg d-slices first so the PE can
            # start as soon as w1[0] + xg[:,0,:] land; biases ride the
            # (otherwise idle at startup) gpsimd DMA queue
            w1q0 = w1_pool.tile([128, DT, 128], F16, tag="w1", name="w1q0")
            nc.sync.dma_start(w1q0, w1t_d[0])
            # graded split: early d-slices land in time for the first it's
            # dt sweep without paying 8 serial descriptor issues
            xg = xg_pool.tile([128, DT, C], F16, tag="xg")
            for a, b in ((0, 1), (1, 2), (2, 4), (4, DT)):
                nc.sync.dma_start(xg[:, a:b, :], xg_d[:, a:b, :])
            b1 = cpool.tile([128, IT_E], F32)
            nc.gpsimd.dma_start(b1, b1_d[:])
            sb1 = cpool.tile([128, IT_S], F32)
            nc.gpsimd.dma_start(sb1, sb1_d[:])

            he = he_pool.tile([128, IT_E, C], F16, tag="he")
            hs = hs_pool.tile([128, IT_S, N], F16, tag="hs")

            # ---- phase A-expert: he = gelu(w1^T.T @ xg^T + b1) ----
            for it in range(IT_E):
                if it == 0:
                    w1q = w1q0
                else:
                    w1q = w1_pool.tile([128, DT, 128], F16, tag="w1")
                    nc.sync.dma_start(w1q, w1t_d[it])
                pcs = [
                    aps.tile([128, ch], F32, tag="aps", name=f"pa{it}_{ci}")
                    for ci, (off, ch) in enumerate(ech)
                ]
                for dt_i in range(DT):
                    for ci, (off, ch) in enumerate(ech):
                        nc.tensor.matmul(
                            pcs[ci],
                            w1q[:, dt_i, :],
                            xg[:, dt_i, off : off + ch],
                            start=(dt_i == 0),
                            stop=(dt_i == DT - 1),
                        )
                for ci, (off, ch) in enumerate(ech):
                    nc.scalar.activation(
                        he[:, it, off : off + ch],
                        pcs[ci],
                        mybir.ActivationFunctionType.Gelu,
                        bias=b1[:, it : it + 1],
                        scale=1.0,
                    )

            # ---- phase A-shared / B-expert / A-shared / B-shared ----
            for half in range(2):
                xs = xs_pool.tile([128, DT, NH], F16, tag="xs")
                nc.sync.dma_start(xs, xsh_d[half])
                for it in range(IT_S):
                    s1q = w1_pool.tile([128, DT, 128], F16, tag="w1")
                    nc.sync.dma_start(s1q, s1t_d[it])
                    pcs = [
                        aps.tile([128, ch], F32, tag="aps", name=f"ps{half}_{it}_{ci}")
                        for ci, (off, ch) in enumerate(sch)
                    ]
                    for dt_i in range(DT):
                        for ci, (off, ch) in enumerate(sch):
                            nc.tensor.matmul(
                                pcs[ci],
                                s1q[:, dt_i, :],
                                xs[:, dt_i, off : off + ch],
                                start=(dt_i == 0),
                                stop=(dt_i == DT - 1),
                            )
                    for ci, (off, ch) in enumerate(sch):
                        nc.scalar.activation(
                            hs[:, it, half * NH + off : half * NH + off + ch],
                            pcs[ci],
                            mybir.ActivationFunctionType.Gelu,
                            bias=sb1[:, it : it + 1],
                            scale=1.0,
                        )

                if half == 0:
                    # ---- phase B-expert: ye = w2^T.T @ he ----
                    for ot in range(DT):
                        w2q = w2_pool.tile([128, IT_E, 128], F16, tag="w2")
                        nc.sync.dma_start(w2q, w2t_d[ot])
                        pys = [
                            bps.tile([128, ch], F32, tag="bps", name=f"pb{ot}_{ci}")
                            for ci, (off, ch) in enumerate(ech)
                        ]
                        for it in range(IT_E):
                            for ci, (off, ch) in enumerate(ech):
                                nc.tensor.matmul(
                                    pys[ci],
                                    w2q[:, it, :],
                                    he[:, it, off : off + ch],
                                    start=(it == 0),
                                    stop=(it == IT_E - 1),
                                )
                        yeb = y_pool.tile([128, C], F16, tag="ye_st", name="yeb")
                        for ci, (off, ch) in enumerate(ech):
                            nc.vector.tensor_copy(yeb[:, off : off + ch], pys[ci])
                        nc.gpsimd.dma_start(ye_d[ot], yeb)

            # ---- phase B-shared: ys = s2^T.T @ hs (partial over i-slice) ----
            for ot in range(DT):
                s2q = s2_pool.tile([128, IT_S, 128], F16, tag="s2")
                nc.sync.dma_start(s2q, s2t_d[ot])
                for half in range(2):
                    pys = [
                        bps.tile([128, ch], F32, tag="bps", name=f"pc{ot}_{half}_{ci}")
                        for ci, (off, ch) in enumerate(sch)
                    ]
                    for it in range(IT_S):
                        for ci, (off, ch) in enumerate(sch):
                            nc.tensor.matmul(
                                pys[ci],
                                s2q[:, it, :],
                                hs[:, it, half * NH + off : half * NH + off + ch],
                                start=(it == 0),
                                stop=(it == IT_S - 1),
                            )
                    ysb = y_pool.tile([128, NH], F16, tag="ys_st", name="ysb")
                    for ci, (off, ch) in enumerate(sch):
                        nc.vector.tensor_copy(ysb[:, off : off + ch], pys[ci])
                    nc.gpsimd.dma_start(
                        ys_d[ot, :, half * NH : (half + 1) * NH], ysb
                    )

    nc.compile()
    return nc


def _get_nc(C):
    if C not in _NC_CACHE:
        _NC_CACHE[C] = build_nc(C)
    return _NC_CACHE[C]


def _route(x, gate_w, top_k):
    """Host gate: fp32 softmax + top-k, replicating the reference.

    Returns (idx_list, w_list): per expert, the token indices routed to it
    and their combine weights (softmax probs of the top-k selection).
    """
    logits = x @ gate_w.T.astype(np.float32)                    # (N, E)
    m = logits.max(axis=1, keepdims=True)
    p = np.exp(logits - m)
    p /= p.sum(axis=1, keepdims=True)                           # (N, E)
    # top-k selection (ties broken by lower index, like jax.lax.top_k)
    order = np.argsort(-p, axis=1, kind="stable")[:, :top_k]    # (N, k)
    idx_list, w_list = [], []
    rows = np.arange(x.shape[0])
    sel_w = p[rows[:, None], order]                             # (N, k)
    for e in range(E):
        mask = (order == e).any(axis=1)
        tok = np.nonzero(mask)[0]
        w = sel_w[mask][order[mask] == e]
        idx_list.append(tok)
        w_list.append(w.astype(np.float32))
    return idx_list, w_list


def _prep_inputs(x16, e_w1, e_b1, e_w2, s_w1, s_b1, s_w2, idx_list, C):
    """Shard + lay out the inputs into the 8 per-core in_maps."""
    xT = np.ascontiguousarray(
        x16.reshape(N, DT, 128).transpose(2, 1, 0)
    )  # (128, DT, N)
    xsh = np.ascontiguousarray(
        np.stack([xT[:, :, :NH], xT[:, :, NH:]])
    )  # (2, 128, DT, NH)

    ew1_16 = np.asarray(e_w1, dtype=np.float16)
    ew2_16 = np.asarray(e_w2, dtype=np.float16)
    sw1_16 = np.asarray(s_w1, dtype=np.float16)
    sw2_16 = np.asarray(s_w2, dtype=np.float16)
    sb1_32 = np.asarray(s_b1, dtype=np.float32)

    in_maps = []
    for e in range(N_CORES):
        tok = idx_list[e]
        xg_rows = np.zeros((C, D), np.float16)
        xg_rows[: len(tok)] = x16[tok]
        xg = np.ascontiguousarray(
            xg_rows.reshape(C, DT, 128).transpose(2, 1, 0)
        )  # (128, DT, C)

        w1t = np.ascontiguousarray(
            ew1_16[e].reshape(IT_E, 128, DT, 128).transpose(0, 3, 2, 1)
        )
        w2t = np.ascontiguousarray(
            ew2_16[e].reshape(DT, 128, IT_E, 128).transpose(0, 3, 2, 1)
        )
        sl = slice(e * IS, (e + 1) * IS)
        s1t = np.ascontiguousarray(
            sw1_16[sl].reshape(IT_S, 128, DT, 128).transpose(0, 3, 2, 1)
        )
        s2t = np.ascontiguousarray(
            sw2_16[:, sl].reshape(DT, 128, IT_S, 128).transpose(0, 3, 2, 1)
        )
        b1c = np.ascontiguousarray(
            np.asarray(e_b1[e], dtype=np.float32).reshape(IT_E, 128).T
        )
        sb1c = np.ascontiguousarray(sb1_32[sl].reshape(IT_S, 128).T)
        in_maps.append(
            {
                "xg": xg,
                "xsh": xsh,
                "w1t": w1t,
                "w2t": w2t,
                "s1t": s1t,
                "s2t": s2t,
                "b1c": b1c,
                "sb1c": sb1c,
            }
        )
    return in_maps


def _gelu_exact(v):
    """erf-based gelu on fp32 numpy, matching jax.nn.gelu(approximate=False)."""
    try:
        from scipy.special import erf
    except ImportError:
        import math

        erf = np.frompyfunc(math.erf, 1, 1)
    return (0.5 * v * (1.0 + erf(v / np.sqrt(2.0)))).astype(np.float32)


# device expert capacity: mean load for top-2-of-8 routing on 4096 tokens.
# Tokens routed beyond an expert's capacity spill to an exact host-side FFN.
CAP = 1024


def run(inputs, trace=False, trace_cores=None):
    """Route on host, run the 8-core Bass kernel, combine on host."""
    x = np.ascontiguousarray(
        np.asarray(inputs["hidden_states"], dtype=np.float32).reshape(N, D)
    )
    gate_w = np.asarray(inputs["gate_w"], dtype=np.float32)
    top_k = int(inputs.get("top_k", 2))

    full_idx, full_w = _route(x, gate_w, top_k)
    C = CAP
    idx_list = [t[:C] for t in full_idx]
    w_list = [w[:C] for w in full_w]
    spill_idx = [t[C:] for t in full_idx]
    spill_w = [w[C:] for w in full_w]

    nc = _get_nc(C)
    x16 = x.astype(np.float16)
    in_maps = _prep_inputs(
        x16, inputs["e_w1"], inputs["e_b1"], inputs["e_w2"],
        inputs["s_w1"], inputs["s_b1"], inputs["s_w2"], idx_list, C,
    )
    if trace:
        install_ntff_hook()
    res = bass_utils.run_bass_kernel_spmd(
        nc,
        in_maps,
        core_ids=list(range(N_CORES)),
        trace=trace,
        trace_cores=trace_cores,
    )

    # ---- host combine (fp32): weighted expert scatter + shared reduce ----
    e_b2 = np.asarray(inputs["e_b2"], dtype=np.float32)
    s_b2 = np.asarray(inputs["s_b2"], dtype=np.float32)
    y = np.zeros((N, D), np.float32)
    for e in range(N_CORES):
        sh = res.results[e]
        tok = idx_list[e]
        ye = sh["ye"].reshape(D, C)[:, : len(tok)].T.astype(np.float32)
        y[tok] += w_list[e][:, None] * (ye + e_b2[e])
        y += sh["ys"].reshape(D, N).T.astype(np.float32)
    y += s_b2

    # ---- capacity spillway: exact host FFN for over-capacity tokens ----
    for e in range(N_CORES):
        tok = spill_idx[e]
        if len(tok) == 0:
            continue
        w1 = np.asarray(inputs["e_w1"][e], dtype=np.float32)
        w2 = np.asarray(inputs["e_w2"][e], dtype=np.float32)
        b1v = np.asarray(inputs["e_b1"][e], dtype=np.float32)
        h = _gelu_exact(x[tok] @ w1.T + b1v)
        y[tok] += spill_w[e][:, None] * (h @ w2.T + e_b2[e])

    out = y.reshape(2, N // 2, D)
    return out, res


def kernel(**inputs):
    out, _ = run(inputs, trace=False)
    return out


# revision 12
# speedup vs baseline: 1.0187x; 1.0187x over previous
"""Trainium2 Bass kernel for nn_MoEBlock_64733747085415.

MoE block: 8 experts (top-2 routed combine) + shared expert, on
B*S = 4096 tokens, D = 1024, I = 4096.

Strategy (expert-parallel with host-side token routing):
  - The gate (softmax over x @ gate_w.T, top-2) is evaluated on the host in
    exact fp32; it only steers the sharding.  Each token's combine weight is
    zero for the 6 unselected experts, so each core only computes its own
    expert's FFN on the ~N*k/E tokens that actually routed to it (gathered,
    zero-padded to capacity C), a 4x FLOP reduction over the dense broadcast.
  - The shared expert is tensor-parallel on the inner dim: core c owns
    i-slice [512c, 512c+512) and processes ALL 4096 tokens; the host sums
    the 8 partial outputs (exact: gelu is elementwise over i).
  - FFN runs feature-major in fp16 (PSUM accumulates fp32): h^T tiles =
    gelu(w1_tile.T @ x^T + b1), y^T tiles = w2_tile.T @ h^T.  Biases b2/s_b2
    and the weighted top-2 combine are applied on the host in fp32.
  - No collectives and no gate on device; the tensor engine runs a pure
    matmul stream (~2048 matmuls/core).

Phase order per core: A-expert, A-shared(half 0), B-expert, A-shared(half 1),
B-shared - so the second half of the replicated x^T streams in behind
B-expert's compute instead of stalling the PE.
"""

import sys
import types

import numpy as np

import concourse.bass as bass
import concourse.mybir as mybir
import concourse.tile as tile
from concourse import bacc
from concourse import bass_utils

F32 = mybir.dt.float32
F16 = mybir.dt.float16

N_CORES = 8
N = 4096          # tokens
D = 1024          # model dim
I = 4096          # expert inner dim
E = 8             # experts
IS = I // N_CORES  # shared-expert inner slice per core (512)
DT = D // 128     # 8 d-tiles
IT_E = I // 128   # 32 expert i-tiles
IT_S = IS // 128  # 4 shared i-tiles
NH = N // 2       # shared-x half (2048 tokens)

_NC_CACHE = {}


def install_ntff_hook():
    """Register the axon NTFF profile hook that boot skips when the antenv
    stub lacks axon_hooks.  Needed only for trace=True runs."""
    if "antenv.axon_hooks" in sys.modules:
        return
    try:
        import trn_agent_boot.trn_boot as tb

        hook = tb._ntff_profile_via_ctypes("/opt/axon/libaxon_pjrt.so")
    except Exception:
        return
    mod = types.ModuleType("antenv.axon_hooks")
    mod.get_axon_ntff_profile_hook = lambda: hook
    mod.set_axon_ntff_profile_hook = lambda h: None
    sys.modules["antenv.axon_hooks"] = mod
    import antenv

    antenv.axon_hooks = mod
    bass_utils.upload_artifacts = lambda tmpdir: tmpdir


def _chunks(total):
    """Split a token count into matmul free-dim chunks of <=512."""
    out = []
    off = 0
    while off < total:
        ch = min(512, total - off)
        out.append((off, ch))
        off += ch
    return out


def build_nc(C):
    nc = bacc.Bacc(
        "TRN2", target_bir_lowering=False, debug=False, num_devices=N_CORES
    )

    # ---- kernel I/O (per-core) ----
    xg_d = nc.dram_tensor("xg", [128, DT, C], F16, kind="ExternalInput")
    xsh_d = nc.dram_tensor("xsh", [2, 128, DT, NH], F16, kind="ExternalInput")
    w1t_d = nc.dram_tensor("w1t", [IT_E, 128, DT, 128], F16, kind="ExternalInput")
    w2t_d = nc.dram_tensor("w2t", [DT, 128, IT_E, 128], F16, kind="ExternalInput")
    s1t_d = nc.dram_tensor("s1t", [IT_S, 128, DT, 128], F16, kind="ExternalInput")
    s2t_d = nc.dram_tensor("s2t", [DT, 128, IT_S, 128], F16, kind="ExternalInput")
    b1_d = nc.dram_tensor("b1c", [128, IT_E], F32, kind="ExternalInput")
    sb1_d = nc.dram_tensor("sb1c", [128, IT_S], F32, kind="ExternalInput")
    ye_d = nc.dram_tensor("ye", [DT, 128, C], F16, kind="ExternalOutput")
    ys_d = nc.dram_tensor("ys", [DT, 128, N], F16, kind="ExternalOutput")

    ech = _chunks(C)    # expert token chunks
    sch = _chunks(NH)   # shared-half token chunks (4 x 512)

    with tile.TileContext(nc) as tc:
        with (
            tc.tile_pool(name="const", bufs=1) as cpool,
            tc.tile_pool(name="xgp", bufs=1) as xg_pool,
            tc.tile_pool(name="xsp", bufs=1) as xs_pool,
            tc.tile_pool(name="hep", bufs=1) as he_pool,
            tc.tile_pool(name="hsp", bufs=1) as hs_pool,
            tc.tile_pool(name="w1s", bufs=6) as w1_pool,
            tc.tile_pool(name="w2s", bufs=2) as w2_pool,
            tc.tile_pool(name="s2s", bufs=2) as s2_pool,
            tc.tile_pool(name="yb", bufs=2) as y_pool,
            tc.tile_pool(name="aps", bufs=4, space="PSUM") as aps,
            tc.tile_pool(name="bps", bufs=4, space="PSUM") as bps,
        ):
            # issue the first weight tile and xg d-slices first so the PE can
            # start as soon as w1[0] + xg[:,0,:] land; biases ride the
            # (otherwise idle at startup) gpsimd DMA queue
            w1q0 = w1_pool.tile([128, DT, 128], F16, tag="w1", name="w1q0")
            nc.sync.dma_start(w1q0, w1t_d[0])
            xg = xg_pool.tile([128, DT, C], F16, tag="xg")
            for dt_i in range(DT):
                nc.sync.dma_start(xg[:, dt_i, :], xg_d[:, dt_i, :])
            b1 = cpool.tile([128, IT_E], F32)
            nc.gpsimd.dma_start(b1, b1_d[:])
            sb1 = cpool.tile([128, IT_S], F32)
            nc.gpsimd.dma_start(sb1, sb1_d[:])

            he = he_pool.tile([128, IT_E, C], F16, tag="he")
            hs = hs_pool.tile([128, IT_S, N], F16, tag="hs")

            # ---- phase A-expert: he = gelu(w1^T.T @ xg^T + b1) ----
            for it in range(IT_E):
                if it == 0:
                    w1q = w1q0
                else:
                    w1q = w1_pool.tile([128, DT, 128], F16, tag="w1")
                    nc.sync.dma_start(w1q, w1t_d[it])
                pcs = [
                    aps.tile([128, ch], F32, tag="aps", name=f"pa{it}_{ci}")
                    for ci, (off, ch) in enumerate(ech)
                ]
                for dt_i in range(DT):
                    for ci, (off, ch) in enumerate(ech):
                        nc.tensor.matmul(
                            pcs[ci],
                            w1q[:, dt_i, :],
                            xg[:, dt_i, off : off + ch],
                            start=(dt_i == 0),
                            stop=(dt_i == DT - 1),
                        )
                for ci, (off, ch) in enumerate(ech):
                    nc.scalar.activation(
                        he[:, it, off : off + ch],
                        pcs[ci],
                        mybir.ActivationFunctionType.Gelu,
                        bias=b1[:, it : it + 1],
                        scale=1.0,
                    )

            # ---- phase A-shared / B-expert / A-shared / B-shared ----
            for half in range(2):
                xs = xs_pool.tile([128, DT, NH], F16, tag="xs")
                nc.sync.dma_start(xs, xsh_d[half])
                for it in range(IT_S):
                    s1q = w1_pool.tile([128, DT, 128], F16, tag="w1")
                    nc.sync.dma_start(s1q, s1t_d[it])
                    pcs = [
                        aps.tile([128, ch], F32, tag="aps", name=f"ps{half}_{it}_{ci}")
                        for ci, (off, ch) in enumerate(sch)
                    ]
                    for dt_i in range(DT):
                        for ci, (off, ch) in enumerate(sch):
                            nc.tensor.matmul(
                                pcs[ci],
                                s1q[:, dt_i, :],
                                xs[:, dt_i, off : off + ch],
                                start=(dt_i == 0),
                                stop=(dt_i == DT - 1),
                            )
                    for ci, (off, ch) in enumerate(sch):
                        nc.scalar.activation(
                            hs[:, it, half * NH + off : half * NH + off + ch],
                            pcs[ci],
                            mybir.ActivationFunctionType.Gelu,
                            bias=sb1[:, it : it + 1],
                            scale=1.0,
                        )

                if half == 0:
                    # ---- phase B-expert: ye = w2^T.T @ he ----
                    for ot in range(DT):
                        w2q = w2_pool.tile([128, IT_E, 128], F16, tag="w2")
                        nc.sync.dma_start(w2q, w2t_d[ot])
                        pys = [
                            bps.tile([128, ch], F32, tag="bps", name=f"pb{ot}_{ci}")
                            for ci, (off, ch) in enumerate(ech)
                        ]
                        for it in range(IT_E):
                            for ci, (off, ch) in enumerate(ech):
                                nc.tensor.matmul(
                                    pys[ci],
                                    w2q[:, it, :],
                                    he[:, it, off : off + ch],
                                    start=(it == 0),
                                    stop=(it == IT_E - 1),
                                )
                        yeb = y_pool.tile([128, C], F16, tag="ye_st", name="yeb")
                        for ci, (off, ch) in enumerate(ech):
                            nc.vector.tensor_copy(yeb[:, off : off + ch], pys[ci])
                        nc.gpsimd.dma_start(ye_d[ot], yeb)

            # ---- phase B-shared: ys = s2^T.T @ hs (partial over i-slice) ----
            for ot in range(DT):
                s2q = s2_pool.tile([128, IT_S, 128], F16, tag="s2")
                nc.sync.dma_start(s2q, s2t_d[ot])
                for half in range(2):
                    pys = [
                        bps.tile([128, ch], F32, tag="bps", name=f"pc{ot}_{half}_{ci}")
                        for ci, (off, ch) in enumerate(sch)
                    ]
                    for it in range(IT_S):
                        for ci, (off, ch) in enumerate(sch):
                            nc.tensor.matmul(
                                pys[ci],
                                s2q[:, it, :],
                                hs[:, it, half * NH + off : half * NH + off + ch],
                                start=(it == 0),
                                stop=(it == IT_S - 1),
                            )
                    ysb = y_pool.tile([128, NH], F16, tag="ys_st", name="ysb")
                    for ci, (off, ch) in enumerate(sch):
                        nc.vector.tensor_copy(ysb[:, off : off + ch], pys[ci])
                    nc.gpsimd.dma_start(
                        ys_d[ot, :, half * NH : (half + 1) * NH], ysb
                    )

    nc.compile()
    return nc


def _get_nc(C):
    if C not in _NC_CACHE:
        _NC_CACHE[C] = build_nc(C)
    return _NC_CACHE[C]


def _route(x, gate_w, top_k):
    """Host gate: fp32 softmax + top-k, replicating the reference.

    Returns (idx_list, w_list): per expert, the token indices routed to it
    and their combine weights (softmax probs of the top-k selection).
    """
    logits = x @ gate_w.T.astype(np.float32)                    # (N, E)
    m = logits.max(axis=1, keepdims=True)
    p = np.exp(logits - m)
    p /= p.sum(axis=1, keepdims=True)                           # (N, E)
    # top-k selection (ties broken by lower index, like jax.lax.top_k)
    order = np.argsort(-p, axis=1, kind="stable")[:, :top_k]    # (N, k)
    idx_list, w_list = [], []
    rows = np.arange(x.shape[0])
    sel_w = p[rows[:, None], order]                             # (N, k)
    for e in range(E):
        mask = (order == e).any(axis=1)
        tok = np.nonzero(mask)[0]
        w = sel_w[mask][order[mask] == e]
        idx_list.append(tok)
        w_list.append(w.astype(np.float32))
    return idx_list, w_list


def _prep_inputs(x16, e_w1, e_b1, e_w2, s_w1, s_b1, s_w2, idx_list, C):
    """Shard + lay out the inputs into the 8 per-core in_maps."""
    xT = np.ascontiguousarray(
        x16.reshape(N, DT, 128).transpose(2, 1, 0)
    )  # (128, DT, N)
    xsh = np.ascontiguousarray(
        np.stack([xT[:, :, :NH], xT[:, :, NH:]])
    )  # (2, 128, DT, NH)

    ew1_16 = np.asarray(e_w1, dtype=np.float16)
    ew2_16 = np.asarray(e_w2, dtype=np.float16)
    sw1_16 = np.asarray(s_w1, dtype=np.float16)
    sw2_16 = np.asarray(s_w2, dtype=np.float16)
    sb1_32 = np.asarray(s_b1, dtype=np.float32)

    in_maps = []
    for e in range(N_CORES):
        tok = idx_list[e]
        xg_rows = np.zeros((C, D), np.float16)
        xg_rows[: len(tok)] = x16[tok]
        xg = np.ascontiguousarray(
            xg_rows.reshape(C, DT, 128).transpose(2, 1, 0)
        )  # (128, DT, C)

        w1t = np.ascontiguousarray(
            ew1_16[e].reshape(IT_E, 128, DT, 128).transpose(0, 3, 2, 1)
        )
        w2t = np.ascontiguousarray(
            ew2_16[e].reshape(DT, 128, IT_E, 128).transpose(0, 3, 2, 1)
        )
        sl = slice(e * IS, (e + 1) * IS)
        s1t = np.ascontiguousarray(
            sw1_16[sl].reshape(IT_S, 128, DT, 128).transpose(0, 3, 2, 1)
        )
        s2t = np.ascontiguousarray(
            sw2_16[:, sl].reshape(DT, 128, IT_S, 128).transpose(0, 3, 2, 1)
        )
        b1c = np.ascontiguousarray(
            np.asarray(e_b1[e], dtype=np.float32).reshape(IT_E, 128).T
        )
        sb1c = np.ascontiguousarray(sb1_32[sl].reshape(IT_S, 128).T)
        in_maps.append(
            {
                "xg": xg,
                "xsh": xsh,
                "w1t": w1t,
                "w2t": w2t,
                "s1t": s1t,
                "s2t": s2t,
                "b1c": b1c,
                "sb1c": sb1c,
            }
        )
    return in_maps


def _gelu_exact(v):
    """erf-based gelu on fp32 numpy, matching jax.nn.gelu(approximate=False)."""
    try:
        from scipy.special import erf
    except ImportError:
        import math

        erf = np.frompyfunc(math.erf, 1, 1)
    return (0.5 * v * (1.0 + erf(v / np.sqrt(2.0)))).astype(np.float32)


# device expert capacity: mean load for top-2-of-8 routing on 4096 tokens.
# Tokens routed beyond an expert's capacity spill to an exact host-side FFN.
CAP = 1024


def run(inputs, trace=False, trace_cores=None):
    """Route on host, run the 8-core Bass kernel, combine on host."""
    x = np.ascontiguousarray(
        np.asarray(inputs["hidden_states"], dtype=np.float32).reshape(N, D)
    )
    gate_w = np.asarray(inputs["gate_w"], dtype=np.float32)
    top_k = int(inputs.get("top_k", 2))

    full_idx, full_w = _route(x, gate_w, top_k)
    C = CAP
    idx_list = [t[:C] for t in full_idx]
    w_list = [w[:C] for w in full_w]
    spill_idx = [t[C:] for t in full_idx]
    spill_w = [w[C:] for w in full_w]

    nc = _get_nc(C)
    x16 = x.astype(np.float16)
    in_maps = _prep_inputs(
        x16, inputs["e_w1"], inputs["e_b1"], inputs["e_w2"],
        inputs["s_w1"], inputs["s_b1"], inputs["s_w2"], idx_list, C,
    )
    if trace:
        install_ntff_hook()
    res = bass_utils.run_bass_kernel_spmd(
        nc,
        in_maps,
        core_ids=list(range(N_CORES)),
        trace=trace,
        trace_cores=trace_cores,
    )

    # ---- host combine (fp32): weighted expert scatter + shared reduce ----
    e_b2 = np.asarray(inputs["e_b2"], dtype=np.float32)
    s_b2 = np.asarray(inputs["s_b2"], dtype=np.float32)
    y = np.zeros((N, D), np.float32)
    for e in range(N_CORES):
        sh = res.results[e]
        tok = idx_list[e]
        ye = sh["ye"].reshape(D, C)[:, : len(tok)].T.astype(np.float32)
        y[tok] += w_list[e][:, None] * (ye + e_b2[e])
        y += sh["ys"].reshape(D, N).T.astype(np.float32)
    y += s_b2

    # ---- capacity spillway: exact host FFN for over-capacity tokens ----
    for e in range(N_CORES):
        tok = spill_idx[e]
        if len(tok) == 0:
            continue
        w1 = np.asarray(inputs["e_w1"][e], dtype=np.float32)
        w2 = np.asarray(inputs["e_w2"][e], dtype=np.float32)
        b1v = np.asarray(inputs["e_b1"][e], dtype=np.float32)
        h = _gelu_exact(x[tok] @ w1.T + b1v)
        y[tok] += spill_w[e][:, None] * (h @ w2.T + e_b2[e])

    out = y.reshape(2, N // 2, D)
    return out, res


def kernel(**inputs):
    out, _ = run(inputs, trace=False)
    return out
